# revision 16
# baseline (speedup 1.0000x reference)
"""Trainium2 Bass kernel for LoRA linear: y = x @ (W + 2*B@A).T + b.

Full inputs: x (8, 2048, 2048) f32, W (2048, 2048) f32, b (2048,) f32,
B (2048, 16) f32, A (16, 2048) f32.  Output (8, 2048, 2048) f32.

Sharding: data-parallel over the batch dim — core i computes
y[i] = x[i] @ w.T + b with the merged weight w = W + 2*B@A.

Per-core kernel (bf16 TensorEngine compute, f32 accumulate):
  ALL transposes are REGULAR matmuls against the identity
  (out = block.T @ I in f32 PSUM — numerically exact for bf16 data).
  is_transpose ops run at a fixed ~107ns/block and each
  transpose<->matmul mode switch costs the PE a ~0.5us pipeline
  flush; regular-matmul transposes issue at the full-clock matmul
  rate (~55ns/block hot) with LDWEIGHTS overlapped and no flushes.

  phase 0 (gpsimd software-DGE queue, ~12us cold start; HWDGE
           dispatch->data is ~7us anyway and only gpsimd can cast):
           A, B staged bf16, x0, bias broadcast, then x1...
  phase W: W row-blocks f32 preloaded on both HWDGE queues (evens
           scalar, odds sync, 6-deep buffer pool); rank-16 delta
           matmul in f32 PSUM; DVE merge w16 = bf16(wrow + delta);
           16 matmul-transposes per block, DVE evicts into wT.
  phase x (interleaved): cast-DMA on gpsimd, 16 matmul-transposes,
           ScalarE evicts.
  main:    per (row tile, 512-col bank): 16 bf16 matmuls into PSUM,
           VectorE adds the bias during eviction, per-bank stores on
           the sync queue.
"""

import numpy as np

import concourse.bacc as bacc
import concourse.mybir as mybir
import concourse.tile as tile
from concourse import masks
from concourse.bass_utils import run_bass_kernel_spmd

N_CORES = 8
BATCH, S, D = 8, 2048, 2048
RANK = 16
SCALE = 2.0  # alpha / rank = 32 / 16
P = 128  # partitions
FREE = 512  # f32 elems per PSUM bank
ND = D // P  # 16 contraction tiles
NS = S // P  # 16 row tiles per core
NO = D // FREE  # 4 output banks per row tile
NG = ND // 4  # 4 groups of 4

F32 = mybir.dt.float32
BF16 = mybir.dt.bfloat16


def build_nc():
    nc = bacc.Bacc(
        "TRN2", target_bir_lowering=False, debug=False, num_devices=N_CORES
    )
    x_d = nc.dram_tensor("x", [S, D], F32, kind="ExternalInput").ap()
    W_d = nc.dram_tensor("W", [D, D], F32, kind="ExternalInput").ap()
    b_d = nc.dram_tensor("b", [D], F32, kind="ExternalInput").ap()
    B_d = nc.dram_tensor("B", [D, RANK], F32, kind="ExternalInput").ap()
    A_d = nc.dram_tensor("A", [RANK, D], F32, kind="ExternalInput").ap()
    out_d = nc.dram_tensor("out", [S, D], F32, kind="ExternalOutput").ap()

    with tile.TileContext(nc) as tc:
        with (
            tc.tile_pool(name="singles", bufs=1) as singles,
            tc.tile_pool(name="wt", bufs=1) as wtp,
        ):
            ident = singles.tile([P, P], BF16)
            masks.make_identity(nc, ident[:])

            A_sb = singles.tile([RANK, D], BF16)
            nc.gpsimd.dma_start(out=A_sb[:], in_=A_d[:])

            B2T = singles.tile([RANK, D], BF16)
            Bs = singles.tile([P, ND * RANK], BF16)
            nc.gpsimd.dma_start(
                out=Bs[:], in_=B_d.rearrange("(t p) r -> p t r", p=P)
            )

            bb = singles.tile([P, D], BF16)

            # merged transposed weight, bf16: wT[p, dt, o] = w[o, dt*128+p]
            wT = wtp.tile([P, ND, D], BF16)

            with (
                tc.tile_pool(name="wrow", bufs=6) as wrowp,
                tc.tile_pool(name="w16", bufs=3) as w16p,
                tc.tile_pool(name="xstage", bufs=4) as xstage,
                tc.tile_pool(name="xTp", bufs=6) as xTp,
                tc.tile_pool(name="yout", bufs=8) as youtp,
                tc.tile_pool(name="dpsum", bufs=3, space="PSUM") as dpsum,
                tc.tile_pool(name="tpsum", bufs=3, space="PSUM") as tpsum,
                tc.tile_pool(name="gpsum", bufs=2, space="PSUM") as gpsum,
            ):
                def x_load(st):
                    xs = xstage.tile([P, D], BF16, tag="xs")
                    nc.gpsimd.dma_start(
                        out=xs[:], in_=x_d[st * P : (st + 1) * P, :]
                    )
                    return xs

                # gpsimd queue order: A, B, x0, bias, x1, x2, ...
                xs0 = x_load(0)
                nc.gpsimd.dma_start(
                    out=bb[:], in_=b_d[None, :].broadcast_to([P, D])
                )
                xs12 = [x_load(1), x_load(2)]

                # W row-blocks preloaded, evens on scalar / odds on sync
                wrows = []
                for ot in range(ND):
                    wrow = wrowp.tile([P, D], F32, tag="wrow")
                    eng = nc.scalar if ot % 2 == 0 else nc.sync
                    eng.dma_start(
                        out=wrow[:], in_=W_d[ot * P : (ot + 1) * P, :]
                    )
                    wrows.append(wrow)

                # 2*B.T: transpose-by-matmul of the staged B tiles
                for g in range(NG):
                    bps = tpsum.tile([RANK, 4 * P], F32, tag="tp")
                    for j in range(4):
                        t = 4 * g + j
                        nc.tensor.matmul(
                            bps[:, j * P : (j + 1) * P],
                            Bs[:, t * RANK : (t + 1) * RANK],
                            ident[:],
                            start=True,
                            stop=True,
                        )
                    nc.vector.tensor_scalar_mul(
                        B2T[:, g * 4 * P : (g + 1) * 4 * P], bps[:], SCALE
                    )

                def x_transpose(xs):
                    # xT[q, dt, s] = x[s, dt*128+q], via out = block.T @ I
                    xT = xTp.tile([P, ND, P], BF16, tag="xT")
                    for g in range(NG):
                        tp = tpsum.tile([P, 4 * P], F32, tag="tp")
                        for j in range(4):
                            dt = 4 * g + j
                            nc.tensor.matmul(
                                tp[:, j * P : (j + 1) * P],
                                xs[:, dt * P : (dt + 1) * P],
                                ident[:],
                                start=True,
                                stop=True,
                            )
                        nc.scalar.copy(xT[:, 4 * g : 4 * (g + 1), :], tp[:])
                    return xT

                # ---- merged-weight build ----
                def w_compute(ot):
                    w16 = w16p.tile([P, D], BF16, tag="w16")
                    dps = [
                        dpsum.tile([P, FREE], F32, tag="dp", name=f"dp{ot}_{g}")
                        for g in range(NG)
                    ]
                    for g in range(NG):
                        nc.tensor.matmul(
                            dps[g][:],
                            B2T[:, ot * P : (ot + 1) * P],
                            A_sb[:, g * FREE : (g + 1) * FREE],
                            start=True,
                            stop=True,
                        )
                    for g in range(NG):
                        nc.vector.tensor_add(
                            w16[:, g * FREE : (g + 1) * FREE],
                            dps[g][:],
                            wrows[ot][:, g * FREE : (g + 1) * FREE],
                        )
                    for g in range(NG):
                        tp = tpsum.tile([P, 4 * P], F32, tag="tp")
                        for j in range(4):
                            dt = 4 * g + j
                            nc.tensor.matmul(
                                tp[:, j * P : (j + 1) * P],
                                w16[:, dt * P : (dt + 1) * P],
                                ident[:],
                                start=True,
                                stop=True,
                            )
                        nc.vector.tensor_scalar_mul(
                            wT[:, 4 * g : 4 * (g + 1), ot * P : (ot + 1) * P],
                            tp[:],
                            1.0,
                        )

                def x_chain(st):
                    return x_transpose(x_load(st))

                xTs = []
                for ot in range(ND):
                    w_compute(ot)
                    if ot == 3:
                        xTs.append(x_transpose(xs0))
                    elif ot == 5:
                        xTs.append(x_transpose(xs12[0]))
                    elif ot == 7:
                        xTs.append(x_transpose(xs12[1]))
                    elif ot in (9, 11, 13):
                        xTs.append(x_chain(len(xTs)))
                PRE = len(xTs)  # 6

                # ---- main loop: y = x @ wT + b ----
                for st in range(NS):
                    if st + PRE < NS:
                        xTs.append(x_chain(st + PRE))
                    xT = xTs[st]
                    for oc in range(NO):
                        gp = gpsum.tile([P, FREE], F32)
                        for dt in range(ND):
                            nc.tensor.matmul(
                                gp[:],
                                xT[:, dt, :],
                                wT[:, dt, oc * FREE : (oc + 1) * FREE],
                                start=(dt == 0),
                                stop=(dt == ND - 1),
                            )
                        ys = youtp.tile([P, FREE], F32, tag="ys")
                        nc.vector.tensor_add(
                            ys[:], gp[:], bb[:, oc * FREE : (oc + 1) * FREE]
                        )
                        nc.sync.dma_start(
                            out=out_d[
                                st * P : (st + 1) * P,
                                oc * FREE : (oc + 1) * FREE,
                            ],
                            in_=ys[:],
                        )

    nc.compile()
    return nc


_NC_CACHE = None


def _get_nc():
    global _NC_CACHE
    if _NC_CACHE is None:
        _NC_CACHE = build_nc()
    return _NC_CACHE


def make_in_maps(x, W, b, B, A):
    x = np.ascontiguousarray(x, dtype=np.float32)
    W = np.ascontiguousarray(W, dtype=np.float32)
    b = np.ascontiguousarray(b, dtype=np.float32)
    B = np.ascontiguousarray(B, dtype=np.float32)
    A = np.ascontiguousarray(A, dtype=np.float32)
    return [
        {"x": x[i], "W": W, "b": b, "B": B, "A": A} for i in range(N_CORES)
    ]


def run(inputs, **spmd_kwargs):
    """Run the SPMD kernel; returns (output, BassKernelResults)."""
    nc = _get_nc()
    in_maps = make_in_maps(**inputs)
    res = run_bass_kernel_spmd(nc, in_maps, core_ids=list(range(N_CORES)), **spmd_kwargs)
    out = np.stack([res.results[i]["out"] for i in range(N_CORES)]).astype(np.float32)
    return out, res


def kernel(x, W, b, B, A):
    out, _ = run({"x": x, "W": W, "b": b, "B": B, "A": A})
    return out


# revision 17
# speedup vs baseline: 1.1551x; 1.1551x over previous
"""Trainium2 Bass kernel for LoRA linear: y = x @ (W + 2*B@A).T + b.

Full inputs: x (8, 2048, 2048) f32, W (2048, 2048) f32, b (2048,) f32,
B (2048, 16) f32, A (16, 2048) f32.  Output (8, 2048, 2048) f32.

Sharding: data-parallel over the batch dim — core i computes
y[i] = x[i] @ w.T + b with the merged weight w = W + 2*B@A.

Per-core kernel (bf16 TensorEngine compute, f32 accumulate):
  The PE is power-throttled to 1.2GHz (HAM k=4) whenever several
  engines + DMA run hot together, and only sustains 2.4GHz on a
  near-pure matmul stream.  A front-loaded weight/transpose phase
  therefore runs at ~half speed no matter how it is scheduled.  So
  the GEMM is restructured OC-OUTER: pass oc sweeps all 16 row
  tiles for one 512-wide output bank and needs only W row-blocks
  4oc..4oc+3; each group's prep (rank-16 delta, DVE merge, 16
  transposes) is emitted sprinkled inside the PREVIOUS pass, keeping
  non-PE engine duty low everywhere.  All 16 transposed x tiles stay
  resident in SBUF (xT 64K/part + wT 64K/part).

  Transposes are regular matmuls against the identity (f32 PSUM,
  numerically exact for bf16 data): they pipeline at the matmul rate
  and avoid is_transpose<->matmul mode switches.

  phase 0 (gpsimd software-DGE queue, ~12us cold start): A, B staged
  bf16, x0, bias broadcast, x1.. cast-loads.  W row-blocks f32 on the
  two HWDGE queues (evens scalar, odds sync).  Per-bank y stores on
  the sync queue.
"""

import numpy as np

import concourse.bacc as bacc
import concourse.mybir as mybir
import concourse.tile as tile
from concourse import masks
from concourse.bass_utils import run_bass_kernel_spmd

N_CORES = 8
BATCH, S, D = 8, 2048, 2048
RANK = 16
SCALE = 2.0  # alpha / rank = 32 / 16
P = 128  # partitions
FREE = 512  # f32 elems per PSUM bank
ND = D // P  # 16 contraction tiles
NS = S // P  # 16 row tiles per core
NO = D // FREE  # 4 output banks per row tile
NG = ND // 4  # 4 groups of 4

F32 = mybir.dt.float32
BF16 = mybir.dt.bfloat16


def build_nc():
    nc = bacc.Bacc(
        "TRN2", target_bir_lowering=False, debug=False, num_devices=N_CORES
    )
    x_d = nc.dram_tensor("x", [S, D], F32, kind="ExternalInput").ap()
    W_d = nc.dram_tensor("W", [D, D], F32, kind="ExternalInput").ap()
    b_d = nc.dram_tensor("b", [D], F32, kind="ExternalInput").ap()
    B_d = nc.dram_tensor("B", [D, RANK], F32, kind="ExternalInput").ap()
    A_d = nc.dram_tensor("A", [RANK, D], F32, kind="ExternalInput").ap()
    out_d = nc.dram_tensor("out", [S, D], F32, kind="ExternalOutput").ap()

    with tile.TileContext(nc) as tc:
        with (
            tc.tile_pool(name="singles", bufs=1) as singles,
            tc.tile_pool(name="wt", bufs=1) as wtp,
            tc.tile_pool(name="xt", bufs=1) as xtp,
        ):
            ident = singles.tile([P, P], BF16)
            masks.make_identity(nc, ident[:])

            A_sb = singles.tile([RANK, D], BF16)
            nc.gpsimd.dma_start(out=A_sb[:], in_=A_d[:])

            B2T = singles.tile([RANK, D], BF16)
            Bs = singles.tile([P, ND * RANK], BF16)
            nc.gpsimd.dma_start(
                out=Bs[:], in_=B_d.rearrange("(t p) r -> p t r", p=P)
            )

            bb = singles.tile([P, D], BF16)

            # both transposed operands fully resident in SBUF
            wT = wtp.tile([P, ND, D], BF16)  # wT[q, dt, o] = w[o, dt*128+q]
            xT = xtp.tile([P, ND, S], BF16)  # xT[q, dt, s] = x[s, dt*128+q]

            with (
                tc.tile_pool(name="wrow", bufs=3) as wrowp,
                tc.tile_pool(name="w16", bufs=2) as w16p,
                tc.tile_pool(name="xstage", bufs=5) as xstage,
                tc.tile_pool(name="yout", bufs=4) as youtp,
                tc.tile_pool(name="dpsum", bufs=3, space="PSUM") as dpsum,
                tc.tile_pool(name="tpsum", bufs=3, space="PSUM") as tpsum,
                tc.tile_pool(name="gpsum", bufs=2, space="PSUM") as gpsum,
            ):
                def x_load(st):
                    xs = xstage.tile([P, D], BF16, tag="xs")
                    nc.gpsimd.dma_start(
                        out=xs[:], in_=x_d[st * P : (st + 1) * P, :]
                    )
                    return xs

                # gpsimd queue order: A, B, x0, bias, x1, x2, ...
                xs_tiles = {0: x_load(0)}
                nc.gpsimd.dma_start(
                    out=bb[:], in_=b_d[None, :].broadcast_to([P, D])
                )
                xs_tiles[1] = x_load(1)
                xs_tiles[2] = x_load(2)

                # W row-blocks, evens on scalar / odds on sync; pool-gated
                # so later groups stream in during the GEMM passes
                wrows = []
                for ot in range(ND):
                    wrow = wrowp.tile([P, D], F32, tag="wrow")
                    eng = nc.scalar if ot % 2 == 0 else nc.sync
                    eng.dma_start(
                        out=wrow[:], in_=W_d[ot * P : (ot + 1) * P, :]
                    )
                    wrows.append(wrow)

                # 2*B.T: transpose-by-matmul of the staged B tiles
                for g in range(NG):
                    bps = tpsum.tile([RANK, 4 * P], F32, tag="tp")
                    for j in range(4):
                        t = 4 * g + j
                        nc.tensor.matmul(
                            bps[:, j * P : (j + 1) * P],
                            Bs[:, t * RANK : (t + 1) * RANK],
                            ident[:],
                            start=True,
                            stop=True,
                        )
                    nc.vector.tensor_scalar_mul(
                        B2T[:, g * 4 * P : (g + 1) * 4 * P], bps[:], SCALE
                    )

                def x_transpose(st):
                    xs = xs_tiles.pop(st) if st in xs_tiles else x_load(st)
                    for g in range(NG):
                        tp = tpsum.tile([P, 4 * P], F32, tag="tp")
                        for j in range(4):
                            dt = 4 * g + j
                            nc.tensor.matmul(
                                tp[:, j * P : (j + 1) * P],
                                xs[:, dt * P : (dt + 1) * P],
                                ident[:],
                                start=True,
                                stop=True,
                            )
                        nc.scalar.copy(
                            xT[:, 4 * g : 4 * (g + 1), st * P : (st + 1) * P],
                            tp[:],
                        )

                def w_compute(ot):
                    w16 = w16p.tile([P, D], BF16, tag="w16")
                    dps = [
                        dpsum.tile([P, FREE], F32, tag="dp", name=f"dp{ot}_{g}")
                        for g in range(NG)
                    ]
                    for g in range(NG):
                        nc.tensor.matmul(
                            dps[g][:],
                            B2T[:, ot * P : (ot + 1) * P],
                            A_sb[:, g * FREE : (g + 1) * FREE],
                            start=True,
                            stop=True,
                        )
                    for g in range(NG):
                        nc.vector.tensor_add(
                            w16[:, g * FREE : (g + 1) * FREE],
                            dps[g][:],
                            wrows[ot][:, g * FREE : (g + 1) * FREE],
                        )
                    for g in range(NG):
                        tp = tpsum.tile([P, 4 * P], F32, tag="tp")
                        for j in range(4):
                            dt = 4 * g + j
                            nc.tensor.matmul(
                                tp[:, j * P : (j + 1) * P],
                                w16[:, dt * P : (dt + 1) * P],
                                ident[:],
                                start=True,
                                stop=True,
                            )
                        nc.vector.tensor_scalar_mul(
                            wT[:, 4 * g : 4 * (g + 1), ot * P : (ot + 1) * P],
                            tp[:],
                            1.0,
                        )

                # group 0 prep + the first x tiles up front
                for ot in range(4):
                    w_compute(ot)
                    x_transpose(ot)

                # ---- oc-outer GEMM: pass oc needs W blocks 4oc..4oc+3;
                # group oc+1 prep and remaining x transposes are sprinkled
                # inside pass oc ----
                nxt_x = 4
                nxt_w = 4
                for oc in range(NO):
                    for st in range(NS):
                        if oc == 0 and st >= 2 and nxt_x < NS:
                            x_transpose(nxt_x)
                            nxt_x += 1
                        if st in (4, 7, 10, 13) and nxt_w < ND:
                            w_compute(nxt_w)
                            nxt_w += 1
                        gp = gpsum.tile([P, FREE], F32)
                        for dt in range(ND):
                            nc.tensor.matmul(
                                gp[:],
                                xT[:, dt, st * P : (st + 1) * P],
                                wT[:, dt, oc * FREE : (oc + 1) * FREE],
                                start=(dt == 0),
                                stop=(dt == ND - 1),
                            )
                        ys = youtp.tile([P, FREE], F32, tag="ys")
                        nc.vector.tensor_add(
                            ys[:], gp[:], bb[:, oc * FREE : (oc + 1) * FREE]
                        )
                        nc.sync.dma_start(
                            out=out_d[
                                st * P : (st + 1) * P,
                                oc * FREE : (oc + 1) * FREE,
                            ],
                            in_=ys[:],
                        )

    nc.compile()
    return nc


_NC_CACHE = None


def _get_nc():
    global _NC_CACHE
    if _NC_CACHE is None:
        _NC_CACHE = build_nc()
    return _NC_CACHE


def make_in_maps(x, W, b, B, A):
    x = np.ascontiguousarray(x, dtype=np.float32)
    W = np.ascontiguousarray(W, dtype=np.float32)
    b = np.ascontiguousarray(b, dtype=np.float32)
    B = np.ascontiguousarray(B, dtype=np.float32)
    A = np.ascontiguousarray(A, dtype=np.float32)
    return [
        {"x": x[i], "W": W, "b": b, "B": B, "A": A} for i in range(N_CORES)
    ]


def run(inputs, **spmd_kwargs):
    """Run the SPMD kernel; returns (output, BassKernelResults)."""
    nc = _get_nc()
    in_maps = make_in_maps(**inputs)
    res = run_bass_kernel_spmd(nc, in_maps, core_ids=list(range(N_CORES)), **spmd_kwargs)
    out = np.stack([res.results[i]["out"] for i in range(N_CORES)]).astype(np.float32)
    return out, res


def kernel(x, W, b, B, A):
    out, _ = run({"x": x, "W": W, "b": b, "B": B, "A": A})
    return out
